# revision 4
# baseline (speedup 1.0000x reference)
"""nn_Attention_19121194402320 on 8 TRN2 NeuronCores.

The reference module is

    k = (key @ Wk.T).reshape(B, H, S, D)       # RAW reshape
    q, v analogously
    attn = softmax(q @ k.T, axis=-1)
    out  = einsum('bnqk,bnvd->bnqd', attn, v)  # NOTE: 'k' vs 'v' labels!
    out.transpose(0,2,1,3).reshape(B, S, E)

The second einsum's contraction labels don't match ('k' in the first
operand, 'v' in the second): einsum semantics sum each independently, so
    out[b,n,q,d] = (sum_k attn[b,n,q,k]) * (sum_v v[b,n,v,d])
                 = sum_v v[b,n,v,d]            (softmax rows sum to 1).
The output is the per-head column-sum of the V projection broadcast over
all query positions; q/k/Wq/Wk do not affect it (verified numerically to
7e-7 against the jax reference).

Per batch b, raw-reshape head h is the contiguous flat chunk
Y[b].flat[h*65536:(h+1)*65536].reshape(1024, 64) of Y = value @ Wv.T, i.e.
chunk g = 12*s + c covers Y[s, 64c:64c+64].  Therefore

    head_sum[h, d] = sum_c  z[h,c,:] @ Wv.T[:, 64c+d]
    z[h,c,:]       = sum_{s in S(h,c)} X[s, :]     (contiguous s-ranges)

computed on-chip as Z.T = Xv.T @ U (U a host-built 0/1 range mask) and 72
small accumulating matmuls against Wv.T.  The full [1024, 384] per-core
output is the 384-vector broadcast, written by the device.

Sharding: core c -> batch c//2, heads 6*(c%2)..6*(c%2)+5, which depend
only on rows [512*(c%2), 512*(c%2+1)) of value[b].  No collectives.
"""

from contextlib import ExitStack

import numpy as np

import concourse.bass as bass
import concourse.tile as tile
from concourse import bacc, mybir
from concourse._compat import with_exitstack
from concourse.bass_utils import run_bass_kernel_spmd

B, S, E, H, D = 4, 1024, 768, 12, 64
SROWS = 512          # value rows per core
HALF = 384           # output columns per core (6 heads * 64)
EC = E // 128        # 6 e-chunks
ST = SROWS // 128    # 4 s-tiles
HL = 6               # heads per core
NU = 72              # mask columns: (c, hl) pairs, index c*6+hl
FP = mybir.dt.float32

_CACHE = {}


def _umask() -> np.ndarray:
    """U[s, c*6+hl] = 1 iff chunk 12*s+c belongs to local head hl."""
    U = np.zeros((SROWS, NU), np.float32)
    for c in range(12):
        for hl in range(HL):
            lo = max(0, (1024 * hl - c + 11) // 12)
            hi = (1024 * (hl + 1) - c + 11) // 12
            U[lo:hi, c * HL + hl] = 1.0
    return U


@with_exitstack
def _body(ctx: ExitStack, tc: tile.TileContext, ins, scr_row, out_ext):
    nc = tc.nc
    pool = ctx.enter_context(tc.tile_pool(name="pool", bufs=1))
    psum = ctx.enter_context(tc.tile_pool(name="psum", bufs=8, space="PSUM"))

    xv = pool.tile([128, ST, E], FP, tag="xv")    # value rows, natural [s, e]
    um = pool.tile([128, ST, NU], FP, tag="um")   # mask, natural [s, u]
    wv = pool.tile([128, EC, E], FP, tag="wv")    # Wv.T, [e, j]
    for st in range(ST):
        nc.sync.dma_start(xv[:, st, :], ins["xv"][st * 128:(st + 1) * 128, :])
        nc.sync.dma_start(um[:, st, :], ins["um"][st * 128:(st + 1) * 128, :])
    for e in range(EC):
        nc.sync.dma_start(wv[:, e, :], ins["wv"][e * 128:(e + 1) * 128, :])

    # Z.T[e, u] = sum_s Xv[s, e] * U[s, u], per 128-wide e-chunk.
    zt = pool.tile([128, EC, NU], FP, tag="zt")
    for e in range(EC):
        pz = psum.tile([128, 512], FP, tag="ps")
        for st in range(ST):
            nc.tensor.matmul(pz[:, 0:NU], xv[:, st, e * 128:(e + 1) * 128],
                             um[:, st, :], start=(st == 0), stop=(st == ST - 1))
        nc.vector.tensor_copy(zt[:, e, :], pz[:, 0:NU])

    # head_sum[hl, d] = sum_c sum_e Z.T[e, c*6+hl] * Wv.T[e, 64c+d]
    po = psum.tile([128, 512], FP, tag="ps")
    n = 0
    for c in range(12):
        for e in range(EC):
            nc.tensor.matmul(po[0:HL, 0:D], zt[:, e, c * HL:(c + 1) * HL],
                             wv[:, e, c * D:(c + 1) * D],
                             start=(n == 0), stop=(n == 12 * EC - 1))
            n += 1
    osb = pool.tile([HL, D], FP, tag="osb")
    nc.vector.tensor_copy(osb, po[0:HL, 0:D])

    # Flatten [6, 64] -> DRAM [384], then broadcast to all 1024 output rows.
    nc.sync.dma_start(scr_row, osb)
    src = bass.AP(tensor=scr_row.tensor, offset=0, ap=[[0, S], [1, HALF]])
    nc.sync.dma_start(out_ext, src)


def _build_nc():
    nc = bacc.Bacc("TRN2", target_bir_lowering=False, debug=False)
    ins = {
        "xv": nc.dram_tensor("xv", [SROWS, E], FP, kind="ExternalInput").ap(),
        "um": nc.dram_tensor("um", [SROWS, NU], FP, kind="ExternalInput").ap(),
        "wv": nc.dram_tensor("wv", [E, E], FP, kind="ExternalInput").ap(),
    }
    out_ext = nc.dram_tensor("out", [S, HALF], FP, kind="ExternalOutput").ap()
    scr_row = nc.dram_tensor("scr_row", [HALF], FP).ap()
    with tile.TileContext(nc) as tc:
        _body(tc, ins, scr_row, out_ext)
    nc.compile()
    return nc


def _get_nc():
    if "nc" not in _CACHE:
        _CACHE["nc"] = _build_nc()
    return _CACHE["nc"]


def _in_maps(inputs):
    v = np.ascontiguousarray(np.asarray(inputs["value"], dtype=np.float32))
    wvT = np.ascontiguousarray(np.asarray(inputs["Wv"], np.float32).T)
    um = _umask()
    maps = []
    for c in range(8):
        b, half = c // 2, c % 2
        rows = slice(half * SROWS, (half + 1) * SROWS)
        maps.append({
            "xv": np.ascontiguousarray(v[b, rows]),
            "um": um,
            "wv": wvT,
        })
    return maps


def _assemble(results):
    out = np.empty((B, S, E), np.float32)
    for c in range(8):
        b, half = c // 2, c % 2
        out[b, :, half * HALF:(half + 1) * HALF] = results[c]["out"]
    return out


def run(inputs, trace=False, **kw):
    """Run on hardware; returns (full_output, BassKernelResults)."""
    nc = _get_nc()
    res = run_bass_kernel_spmd(nc, _in_maps(inputs), core_ids=list(range(8)),
                               trace=trace, **kw)
    return _assemble(res.results), res


def kernel(**inputs) -> np.ndarray:
    out, _ = run(inputs)
    return out


# revision 8
# speedup vs baseline: 1.6176x; 1.6176x over previous
"""nn_Attention_19121194402320 on 8 TRN2 NeuronCores.

The reference module is

    k = (key @ Wk.T).reshape(B, H, S, D)       # RAW reshape
    q, v analogously
    attn = softmax(q @ k.T, axis=-1)
    out  = einsum('bnqk,bnvd->bnqd', attn, v)  # NOTE: 'k' vs 'v' labels!
    out.transpose(0,2,1,3).reshape(B, S, E)

The second einsum's contraction labels don't match ('k' in the first
operand, 'v' in the second): einsum semantics sum each independently, so
    out[b,n,q,d] = (sum_k attn[b,n,q,k]) * (sum_v v[b,n,v,d])
                 = sum_v v[b,n,v,d]            (softmax rows sum to 1).
The output is the per-head column-sum of the V projection broadcast over
all query positions; q/k/Wq/Wk do not affect it (verified numerically to
7e-7 against the jax reference).

Per batch b, raw-reshape head h is the contiguous flat chunk
Y[b].flat[h*65536:(h+1)*65536].reshape(1024, 64) of Y = value @ Wv.T, i.e.
chunk g = 12*s + c covers Y[s, 64c:64c+64].  Therefore

    head_sum[h, d] = sum_c  z[h,c,:] @ Wv.T[:, 64c+d]
    z[h,c,:]       = sum_{s in S(h,c)} X[s, :]     (contiguous s-ranges)

computed on-chip as Z.T = Xv.T @ U (U a host-built 0/1 range mask) and 72
small accumulating matmuls against Wv.T.  The full [1024, 384] per-core
output is the 384-vector broadcast, written by the device.

Sharding: core c -> batch c//2, heads 6*(c%2)..6*(c%2)+5, which depend
only on rows [512*(c%2), 512*(c%2+1)) of value[b].  No collectives.
"""

from contextlib import ExitStack

import numpy as np

import concourse.bass as bass
import concourse.tile as tile
from concourse import bacc, mybir
from concourse._compat import with_exitstack
from concourse.bass_utils import run_bass_kernel_spmd

B, S, E, H, D = 4, 1024, 768, 12, 64
SROWS = 512          # value rows per core
HALF = 384           # output columns per core (6 heads * 64)
EC = E // 128        # 6 e-chunks
ST = SROWS // 128    # 4 s-tiles
HL = 6               # heads per core
NU = 72              # mask columns: (c, hl) pairs, index c*6+hl
FP = mybir.dt.float32

_CACHE = {}


def _umask() -> np.ndarray:
    """U[s, c*6+hl] = 1 iff chunk 12*s+c belongs to local head hl."""
    U = np.zeros((SROWS, NU), np.float32)
    for c in range(12):
        for hl in range(HL):
            lo = max(0, (1024 * hl - c + 11) // 12)
            hi = (1024 * (hl + 1) - c + 11) // 12
            U[lo:hi, c * HL + hl] = 1.0
    return U


@with_exitstack
def _body(ctx: ExitStack, tc: tile.TileContext, ins, scr_row, out_ext):
    nc = tc.nc
    FR = mybir.dt.float32r
    pool = ctx.enter_context(tc.tile_pool(name="pool", bufs=1))
    psum = ctx.enter_context(tc.tile_pool(name="psum", bufs=8, space="PSUM"))

    xv = pool.tile([128, ST, E], FR, tag="xv")    # value rows, natural [s, e]
    um = pool.tile([128, ST, NU], FR, tag="um")   # mask, natural [s, u]
    wv = pool.tile([128, EC, E], FR, tag="wv")    # Wv.T, [e, j]
    for st in range(ST):
        nc.sync.dma_start(xv[:, st, :], ins["xv"][st * 128:(st + 1) * 128, :])
        nc.sync.dma_start(um[:, st, :], ins["um"][st * 128:(st + 1) * 128, :])
    for e in range(EC):
        nc.sync.dma_start(wv[:, e, :], ins["wv"][e * 128:(e + 1) * 128, :])

    # Z.T[e, u] = sum_s Xv[s, e] * U[s, u], per 128-wide e-chunk.
    zt = pool.tile([128, EC, NU], FR, tag="zt")
    for e in range(EC):
        pz = psum.tile([128, 512], FP, tag="ps")
        for st in range(ST):
            nc.tensor.matmul(pz[:, 0:NU],
                             xv[:, st, e * 128:(e + 1) * 128], um[:, st, :],
                             start=(st == 0), stop=(st == ST - 1))
        nc.vector.tensor_copy(zt[:, e, :], pz[:, 0:NU])

    # Full G[u, j] = sum_e Z.T[e, u] * Wv.T[e, j] in two PSUM banks; the
    # N>=256 moving dim keeps float32r on its fast path.  Only the twelve
    # diagonal blocks G[c*6:c*6+6, 64c:64c+64] are needed; DVE folds them.
    pga = psum.tile([128, 512], FP, tag="ps")
    pgb = psum.tile([128, 512], FP, tag="ps")
    for e in range(EC):
        zsl = zt[:, e, :]
        nc.tensor.matmul(pga[0:NU, :], zsl, wv[:, e, 0:512],
                         start=(e == 0), stop=(e == EC - 1))
        nc.tensor.matmul(pgb[0:NU, 0:256], zsl, wv[:, e, 512:768],
                         start=(e == 0), stop=(e == EC - 1))
    # Compute-engine APs must start at partition 0/32/64/96, so the
    # 6-partition-aligned diagonal blocks are gathered with DMAs instead:
    # G -> SBUF, 12 block DMAs into [6, 64, 12] (c innermost), one reduce.
    gsb = pool.tile([128, E], FP, tag="gsb")
    nc.vector.tensor_copy(gsb[0:NU, 0:512], pga[0:NU, :])
    nc.vector.tensor_copy(gsb[0:NU, 512:768], pgb[0:NU, 0:256])
    dstack = pool.tile([HL, D, 12], FP, tag="dstack")
    for c in range(12):
        nc.sync.dma_start(dstack[:, :, c],
                          gsb[c * HL:(c + 1) * HL, c * D:(c + 1) * D])
    osb = pool.tile([HL, D], FP, tag="osb")
    nc.vector.reduce_sum(osb, dstack, axis=mybir.AxisListType.X)

    # Flatten [6, 64] -> DRAM [384]; broadcast-load to a [128, 384] SBUF
    # tile; one big SBUF->DRAM write replicates it over all 1024 rows.
    nc.sync.dma_start(scr_row, osb)
    bc = pool.tile([128, HALF], FP, tag="bc")
    row_bcast = bass.AP(tensor=scr_row.tensor, offset=0, ap=[[0, 128], [1, HALF]])
    nc.sync.dma_start(bc, row_bcast)
    src = bass.AP(tensor=bc.tensor, offset=bc.offset,
                  ap=[bc.ap[0], [0, S // 128], bc.ap[1]])
    dst = out_ext.rearrange("(ot t) j -> t ot j", t=128)
    nc.sync.dma_start(dst, src)


def _build_nc():
    nc = bacc.Bacc("TRN2", target_bir_lowering=False, debug=False)
    FR = mybir.dt.float32r
    ins = {
        "xv": nc.dram_tensor("xv", [SROWS, E], FR, kind="ExternalInput").ap(),
        "um": nc.dram_tensor("um", [SROWS, NU], FR, kind="ExternalInput").ap(),
        "wv": nc.dram_tensor("wv", [E, E], FR, kind="ExternalInput").ap(),
    }
    out_ext = nc.dram_tensor("out", [S, HALF], FP, kind="ExternalOutput").ap()
    scr_row = nc.dram_tensor("scr_row", [HALF], FP).ap()
    with tile.TileContext(nc) as tc:
        _body(tc, ins, scr_row, out_ext)
    nc.compile()
    return nc


def _get_nc():
    if "nc" not in _CACHE:
        _CACHE["nc"] = _build_nc()
    return _CACHE["nc"]


def _in_maps(inputs):
    v = np.ascontiguousarray(np.asarray(inputs["value"], dtype=np.float32))
    wvT = np.ascontiguousarray(np.asarray(inputs["Wv"], np.float32).T)
    um = _umask()
    maps = []
    for c in range(8):
        b, half = c // 2, c % 2
        rows = slice(half * SROWS, (half + 1) * SROWS)
        maps.append({
            "xv": np.ascontiguousarray(v[b, rows]),
            "um": um,
            "wv": wvT,
        })
    return maps


def _assemble(results):
    out = np.empty((B, S, E), np.float32)
    for c in range(8):
        b, half = c // 2, c % 2
        out[b, :, half * HALF:(half + 1) * HALF] = results[c]["out"]
    return out


def run(inputs, trace=False, **kw):
    """Run on hardware; returns (full_output, BassKernelResults)."""
    nc = _get_nc()
    res = run_bass_kernel_spmd(nc, _in_maps(inputs), core_ids=list(range(8)),
                               trace=trace, **kw)
    return _assemble(res.results), res


def kernel(**inputs) -> np.ndarray:
    out, _ = run(inputs)
    return out
